# revision 36
# baseline (speedup 1.0000x reference)
"""Trainium2 Bass kernel for sliding-window Pearson correlation attention.

Input  x: [512, 2, 32768] f32.
Output attentions: [512, 32669] f32 = relu(corr - mean_b(corr)) where corr is
the per-batch sliding-window (w=100) Pearson correlation of the two channels.

Sharding: split the T/output dimension across the 8 cores (4084 output
columns each, + 99-column halo on the input). Every core sees all 512
batches, so the batch-mean is computed locally - no collective needed.

The T range is cut into 5 independent column chunks; each (chunk, batch
tile) task is software-pipelined with a one-stage skew and 2-deep DMA
prefetch so the in-order engine queues never stall behind the cross-engine
post chain. Per task:
  scans   DVE   5 windowed sums via tensor_tensor_scan (f32 in, bf16 out;
                s12/u in f32), seeded per chunk by an f32 reduce over the
                100-col halo (leading zero column for chunk 0)
  squares Act   e = w*x^2, t = s^2 (scale folded into the activation)
  x1s     A/P   w*x1 (split Act copy-scale / Pool tensor_scalar)
  e12     Pool  x1s*x2 tensor_tensor
  u       Pool  s1*s2 (f32 out - Pool rates are dtype-blind)
  v1,v2   PE    identity-matmul accumulate: v = I@s11 - I@t into PSUM
                (weights are free, so the subtract rides the matmul)
  r12     Act   Abs_reciprocal_sqrt(v) from PSUM - one act table covers
                Square/Copy/Relu/AbsRsqrt so only one table load total
  cov     Pool  s12 - u ; m0 = r1*r2 and corr = cov*m0 on DVE (bf16 2x)
  mean    PE    (1/B)ones-matmul accumulated over the 4 batch tiles
  fsub    PE    I@corr - broadcast(avg) into PSUM
  relu    A/D   psum -> sbuf f32 (Act Relu / DVE max, interleaved), DMA out
Hardware-ISA notes baked in: Pool/GPSIMD cannot touch PSUM and rejects
scalar_tensor_tensor and tensor_tensor divide/max; there is no TT divide on
any engine (hence rsqrt+multiplies); fp32r matmul needs rounded producers.
"""

import numpy as np

import concourse.bass as bass
import concourse.mybir as mybir
import concourse.tile as tile
from concourse.bass_utils import run_bass_kernel_spmd

WIN = 100
B = 512
CH = 2
T = 32768
N = T - WIN + 1  # 32669
NCORES = 8
NLOC = 4084  # output columns per core (8*4084 = 32672 >= N; tail dropped)
FIN = NLOC + WIN - 1  # 4183 input columns per core
TPAD = (NCORES - 1) * NLOC + FIN  # 32771 (input padded with 3 zero cols)
P = 128
NBT = B // P  # 4 batch tiles

CHUNKS = [511, 1021, 1021, 1020, 511]  # scan chunk widths along T (sum = NLOC)
assert sum(CHUNKS) == NLOC
COFF = [sum(CHUNKS[:i]) for i in range(len(CHUNKS))]
NSC = len(CHUNKS)
FSMAX = max(CHUNKS)
WTMAX = FSMAX + WIN + 1  # input cols per scan chunk (incl. leading zero/halo)

SL = 512  # psum-bank slice for matmuls
NSL = (NLOC + SL - 1) // SL  # 8 slices (last = 500)

f32 = mybir.dt.float32
bf16 = mybir.dt.bfloat16
AOT = mybir.ActivationFunctionType
ALU = mybir.AluOpType
AXL = mybir.AxisListType

# engine-split knobs (fraction of columns handled by the DVE engine; the
# rest goes to Pool for SBUF ops / Act for PSUM-reading relu)
U_DVE = 0.0      # u = s1*s2
M0_DVE = 1.0     # m0 = r1*r2
COV_DVE = 0.0    # cov = s12w - u
CORR_DVE = 1.0   # corr = cov*m0
RELU_ACT = 0.65  # relu: Act share (rest DVE tensor_scalar_max; Pool can't read PSUM)
X1S_ACT = 0.5    # x1s = w*x1: Act share (rest Pool tensor_scalar)
PSL = 1021       # slice width for Pool/DVE post ops (latency hiding)


def _slices(total, step):
    return [(i, min(i + step, total)) for i in range(0, total, step)]


def _kernel_body(tc, out, xs, wconst):
    nc = tc.nc
    import contextlib

    ctx = contextlib.ExitStack()
    with ctx:
        const_pool = ctx.enter_context(tc.tile_pool(name="const", bufs=1))
        in_pool = ctx.enter_context(tc.tile_pool(name="scanin", bufs=4))
        s_pool = ctx.enter_context(tc.tile_pool(name="scanout", bufs=2))
        post_pool = ctx.enter_context(tc.tile_pool(name="post", bufs=2))
        corr_pool = ctx.enter_context(tc.tile_pool(name="corrp", bufs=2))
        row_pool = ctx.enter_context(tc.tile_pool(name="rows", bufs=2))
        out_pool = ctx.enter_context(tc.tile_pool(name="outp", bufs=4))
        v_psum = ctx.enter_context(tc.tile_pool(name="vps", bufs=2, space="PSUM"))
        b_psum = ctx.enter_context(tc.tile_pool(name="bps", bufs=1, space="PSUM"))
        f_psum = ctx.enter_context(tc.tile_pool(name="fps", bufs=2, space="PSUM"))

        # constants: I and -I (bf16, exact); DMAs issued after the first
        # input prefetches (see below) so they don't delay the pipeline head
        identP = const_pool.tile([P, P], bf16, tag="identP")
        identN = const_pool.tile([P, P], bf16, tag="identN")
        ident = [identP, identN]
        bcol = const_pool.tile([P, 1], bf16, tag="bcol")
        nc.vector.memset(bcol[:], 1.0 / B)
        negrow = const_pool.tile([1, P], bf16, tag="negrow")
        nc.vector.memset(negrow[:], -1.0)

        SQW = float(np.sqrt(WIN))

        # column-major: each T-chunk is fully independent (scan seeds come
        # from a reduce over the chunk's own halo). The (c, bt) tasks are
        # software-pipelined with a one-stage skew so every engine queue sees
        # task k+1's scan-stage ops before task k's post-stage ops — the
        # in-order queues never stall behind the cross-engine post chain.
        def dma_stage(c, bt):
            fs = CHUNKS[c]
            c0 = COFF[c]
            wt = fs + WIN + 1
            b0 = bt * P
            x12z = in_pool.tile([P, CH, WTMAX], f32, tag="x12z")
            if c == 0:
                # leading zero column so data1[0] reads 0 at t=0
                nc.vector.memset(x12z[:, :, 0:1], 0.0)
                nc.sync.dma_start(
                    out=x12z[:, :, 1:wt],
                    in_=xs[b0 : b0 + P, :, 0 : wt - 1],
                )
            else:
                # halo: col j holds x[c0-1+j]
                nc.sync.dma_start(
                    out=x12z[:, :, 0 : wt - 1],
                    in_=xs[b0 : b0 + P, :, c0 - 1 : c0 - 1 + wt - 1],
                )
            return x12z

        def scan_stage(c, bt, x12z):
            fs = CHUNKS[c]
            wt = fs + WIN + 1
            e = in_pool.tile([P, CH, WTMAX], bf16, tag="e")
            nc.scalar.activation(e[:, :, 0:wt], x12z[:, :, 0:wt], AOT.Square, scale=SQW)
            x1s = in_pool.tile([P, WTMAX], bf16, tag="x1s")
            xcut = int(wt * X1S_ACT)
            if xcut > 0:
                nc.scalar.mul(x1s[:, 0:xcut], x12z[:, 0, 0:xcut], float(WIN))
            if xcut < wt:
                nc.gpsimd.tensor_scalar_mul(
                    x1s[:, xcut:wt], x12z[:, 0, xcut:wt], float(WIN)
                )
            e12 = in_pool.tile([P, WTMAX], bf16, tag="e12")
            nc.gpsimd.tensor_mul(e12[:, 0:wt], x1s[:, 0:wt], x12z[:, 1, 0:wt])

            s_pair = s_pool.tile([P, CH, FSMAX], bf16, tag="s_pair")
            se_pair = s_pool.tile([P, CH, FSMAX], bf16, tag="se_pair")
            s12 = s_pool.tile([P, FSMAX], f32, tag="s12")

            # seed reduces over the halo cols [0, WIN): one per source tile
            # (both channels at once for the pair tiles)
            init_x = in_pool.tile([P, CH], f32, tag="init_x")
            nc.vector.reduce_sum(init_x[:], x12z[:, :, 0:WIN], axis=AXL.X)
            init_e = in_pool.tile([P, CH], f32, tag="init_e")
            nc.vector.reduce_sum(init_e[:], e[:, :, 0:WIN], axis=AXL.X)
            init_12 = in_pool.tile([P, 1], f32, tag="init_12")
            nc.vector.reduce_sum(init_12[:], e12[:, 0:WIN], axis=AXL.X)

            def wsum(dst, src, ch, init):
                # dst[:, t] = windowed sum at c0+t; src col j = a[c0-1+j]
                # (j=0 is a zero col for chunk 0). Seed = window at t-1.
                if ch is not None:
                    d0, d1 = src[:, ch, WIN : WIN + fs], src[:, ch, 0:fs]
                else:
                    d0, d1 = src[:, WIN : WIN + fs], src[:, 0:fs]
                nc.vector.tensor_tensor_scan(
                    out=dst,
                    data0=d0,
                    data1=d1,
                    initial=init,
                    op0=ALU.add,
                    op1=ALU.subtract,
                )

            wsum(s_pair[:, 0, 0:fs], x12z, 0, init_x[:, 0:1])
            wsum(s_pair[:, 1, 0:fs], x12z, 1, init_x[:, 1:2])
            wsum(se_pair[:, 0, 0:fs], e, 0, init_e[:, 0:1])
            wsum(se_pair[:, 1, 0:fs], e, 1, init_e[:, 1:2])
            wsum(s12[:, 0:fs], e12, None, init_12[:])
            return s_pair, se_pair, s12

        def split_op(dve_emit, pool_emit, frac, fs):
            # column-split an elementwise op: [0, cut) on DVE, [cut, fs) Pool
            cut = int(fs * frac) // 2 * 2
            for (l, r) in _slices(fs, PSL):
                dl, dr = min(l, cut), min(r, cut)
                if dl < dr:
                    dve_emit(dl, dr)
                pl, pr = max(l, cut), max(r, cut)
                if pl < pr:
                    pool_emit(pl, pr)

        def post_stage(c, bt, scans):
            fs = CHUNKS[c]
            s_pair, se_pair, s12 = scans
            # t = s^2 (plain; the w-scaling lives in e/e12)
            t_pair = post_pool.tile([P, CH, FSMAX], bf16, tag="t_pair")
            nc.scalar.activation(
                t_pair[:, :, 0:fs], s_pair[:, :, 0:fs], AOT.Square
            )
            u = post_pool.tile([P, FSMAX], f32, tag="u")
            split_op(
                lambda l, r: nc.vector.tensor_mul(
                    u[:, l:r], s_pair[:, 0, l:r], s_pair[:, 1, l:r]
                ),
                lambda l, r: nc.gpsimd.tensor_mul(
                    u[:, l:r], s_pair[:, 0, l:r], s_pair[:, 1, l:r]
                ),
                U_DVE, fs,
            )

            # v = w*s11 - s1^2 (PE identity matmuls into PSUM), rsqrt on Act
            r_pair = post_pool.tile([P, CH, FSMAX], bf16, tag="t_pair", name="r_pair")
            for (l, r) in _slices(fs, SL):
                vps = v_psum.tile([P, CH, SL], f32, tag="vps")
                for ch in range(CH):
                    nc.tensor.matmul(
                        vps[:, ch, 0 : r - l], ident[0][:], se_pair[:, ch, l:r],
                        start=True, stop=False,
                    )
                    nc.tensor.matmul(
                        vps[:, ch, 0 : r - l], ident[1][:], t_pair[:, ch, l:r],
                        start=False, stop=True,
                    )
                nc.scalar.activation(
                    r_pair[:, :, l:r], vps[:, :, 0 : r - l], AOT.Abs_reciprocal_sqrt
                )

            # cov = w*s12 - u ; m0 = r1*r2 ; corr = cov*m0
            cov = post_pool.tile([P, FSMAX], bf16, tag="cov")
            split_op(
                lambda l, r: nc.vector.tensor_sub(cov[:, l:r], s12[:, l:r], u[:, l:r]),
                lambda l, r: nc.gpsimd.tensor_sub(cov[:, l:r], s12[:, l:r], u[:, l:r]),
                COV_DVE, fs,
            )
            # reuses u's slot (u is dead after cov) to stay inside SBUF
            m0 = post_pool.tile([P, FSMAX], bf16, tag="u", name="m0")
            split_op(
                lambda l, r: nc.vector.tensor_mul(
                    m0[:, l:r], r_pair[:, 0, l:r], r_pair[:, 1, l:r]
                ),
                lambda l, r: nc.gpsimd.tensor_mul(
                    m0[:, l:r], r_pair[:, 0, l:r], r_pair[:, 1, l:r]
                ),
                M0_DVE, fs,
            )
            corr = corr_pool.tile([P, FSMAX], bf16, tag=f"corr{bt}")
            split_op(
                lambda l, r: nc.vector.tensor_mul(corr[:, l:r], cov[:, l:r], m0[:, l:r]),
                lambda l, r: nc.gpsimd.tensor_mul(corr[:, l:r], cov[:, l:r], m0[:, l:r]),
                CORR_DVE, fs,
            )
            return corr

        def mean_store_stage(c, corrs):
            fs = CHUNKS[c]
            c0 = COFF[c]
            srow = row_pool.tile([1, FSMAX], bf16, tag="srow")
            for (l, r) in _slices(fs, SL):
                bps = b_psum.tile([1, SL], f32, tag="bps")
                for bt in range(NBT):
                    nc.tensor.matmul(
                        bps[:, 0 : r - l], bcol[:], corrs[bt][:, l:r],
                        start=(bt == 0), stop=(bt == NBT - 1),
                    )
                nc.scalar.activation(srow[:, l:r], bps[:, 0 : r - l], AOT.Copy)

            half = (fs + 1) // 2
            hi = 0
            for bt in range(NBT):
                b0 = bt * P
                for h0 in range(0, fs, half):
                    h1 = min(h0 + half, fs)
                    outt = out_pool.tile([P, (FSMAX + 1) // 2], f32, tag="outt")
                    fps = f_psum.tile([P, (FSMAX + 1) // 2], f32, tag="fps")
                    for (l, r) in _slices(h1 - h0, SL):
                        l, r = l + h0, r + h0
                        nc.tensor.matmul(
                            fps[:, l - h0 : r - h0], ident[0][:], corrs[bt][:, l:r],
                            start=True, stop=False,
                        )
                        nc.tensor.matmul(
                            fps[:, l - h0 : r - h0], negrow[:], srow[:, l:r],
                            start=False, stop=True,
                        )
                    if int((hi + 1) * RELU_ACT) > int(hi * RELU_ACT):
                        nc.scalar.activation(
                            outt[:, 0 : h1 - h0], fps[:, 0 : h1 - h0], AOT.Relu
                        )
                    else:
                        nc.vector.tensor_scalar_max(
                            outt[:, 0 : h1 - h0], fps[:, 0 : h1 - h0], 0.0
                        )
                    hi += 1
                    nc.sync.dma_start(
                        out=out[b0 : b0 + P, c0 + h0 : c0 + h1],
                        in_=outt[:, 0 : h1 - h0],
                    )

        tasks = [(c, bt) for c in range(NSC) for bt in range(NBT)]
        xq = {}
        scans_q = {}
        corrs_q = {}
        for j in range(min(2, len(tasks))):
            xq[j] = dma_stage(*tasks[j])
        nc.sync.dma_start(out=identP[:], in_=wconst[0, :, :])
        nc.sync.dma_start(out=identN[:], in_=wconst[1, :, :])
        for i in range(len(tasks) + 1):
            if i + 2 < len(tasks):
                xq[i + 2] = dma_stage(*tasks[i + 2])
            if i < len(tasks):
                scans_q[tasks[i]] = scan_stage(*tasks[i], xq.pop(i))
            if i > 0:
                c, bt = tasks[i - 1]
                corrs_q[(c, bt)] = post_stage(c, bt, scans_q.pop(tasks[i - 1]))
                if bt == NBT - 1:
                    mean_store_stage(c, [corrs_q.pop((c, b)) for b in range(NBT)])


def build_nc():
    from concourse import bacc

    nc = bacc.Bacc("TRN2", target_bir_lowering=False, debug=False, num_devices=NCORES)
    xs = nc.dram_tensor("xs", [B, CH, FIN], f32, kind="ExternalInput").ap()
    wconst = nc.dram_tensor("wconst", [CH, P, P], bf16, kind="ExternalInput").ap()
    out = nc.dram_tensor("out", [B, NLOC], f32, kind="ExternalOutput").ap()
    with tile.TileContext(nc) as tc:
        _kernel_body(tc, out, xs, wconst)
    nc.compile()
    return nc


_NC = None


def _get_nc():
    global _NC
    if _NC is None:
        _NC = build_nc()
    return _NC


def make_in_maps(x):
    import ml_dtypes

    x = np.asarray(x, dtype=np.float32)
    xpad = np.zeros((B, CH, TPAD), dtype=np.float32)
    xpad[:, :, :T] = x
    eye = np.eye(P, dtype=np.float32)
    wconst = np.stack([eye, -eye]).astype(ml_dtypes.bfloat16)
    return [
        {
            "xs": np.ascontiguousarray(xpad[:, :, c * NLOC : c * NLOC + FIN]),
            "wconst": wconst,
        }
        for c in range(NCORES)
    ]


def _run(x, **kwargs):
    nc = _get_nc()
    res = run_bass_kernel_spmd(nc, make_in_maps(x), core_ids=list(range(NCORES)), **kwargs)
    outs = [np.asarray(res.results[c]["out"]) for c in range(NCORES)]
    full = np.concatenate(outs, axis=1)[:, :N].astype(np.float32)
    return full, res


def kernel(x):
    full, _ = _run(x)
    return full


# revision 54
# speedup vs baseline: 1.0434x; 1.0434x over previous
"""Trainium2 Bass kernel for sliding-window Pearson correlation attention.

Input  x: [512, 2, 32768] f32.
Output attentions: [512, 32669] f32 = relu(corr - mean_b(corr)) where corr is
the per-batch sliding-window (w=100) Pearson correlation of the two channels.

Sharding: split the T/output dimension across the 8 cores (4084 output
columns each, + 99-column halo on the input). Every core sees all 512
batches, so the batch-mean is computed locally - no collective needed.

The T range is cut into 4 independent column chunks; each (chunk, batch
tile) task is software-pipelined with a one-stage skew and 2-deep DMA
prefetch so the in-order engine queues never stall behind the cross-engine
post chain. Per task:
  scans   DVE   5 windowed sums via tensor_tensor_scan (f32 in, bf16 out;
                s12/u in f32). Self-seeding: each input tile carries a
                100-col zero prefix (zeroed once per buffer), so the scan
                builds its opening window from initial=0 - no seed reduces
  squares Act   e = w*x^2, t = s^2 (scale folded into the activation)
  x1s     A/P   w*x1 (split Act copy-scale / Pool tensor_scalar)
  e12     Pool  x1s*x2 tensor_tensor
  u       Pool  s1*s2 (f32 out - Pool rates are dtype-blind)
  v1,v2   PE    identity-matmul accumulate: v = I@s11 - I@t into PSUM
                (weights are free, so the subtract rides the matmul)
  r12     Act   Abs_reciprocal_sqrt(v) from PSUM - one act table covers
                Square/Copy/Relu/AbsRsqrt so only one table load total
  cov     Pool  s12 - u ; m0 = r1*r2 and corr = cov*m0 on DVE (bf16 2x)
  mean    PE    (1/B)ones-matmul accumulated over the 4 batch tiles
  fsub    PE    I@corr - broadcast(avg) into PSUM
  relu    A/D   psum -> sbuf f32 (Act Relu / DVE max, interleaved), DMA out
Hardware-ISA notes baked in: Pool/GPSIMD cannot touch PSUM and rejects
scalar_tensor_tensor and tensor_tensor divide/max; there is no TT divide on
any engine (hence rsqrt+multiplies); fp32r matmul needs rounded producers.
"""

import numpy as np

import concourse.bass as bass
import concourse.mybir as mybir
import concourse.tile as tile
from concourse.bass_utils import run_bass_kernel_spmd

WIN = 100
B = 512
CH = 2
T = 32768
N = T - WIN + 1  # 32669
NCORES = 8
NLOC = 4084  # output columns per core (8*4084 = 32672 >= N; tail dropped)
FIN = NLOC + WIN - 1  # 4183 input columns per core
TPAD = (NCORES - 1) * NLOC + FIN  # 32771 (input padded with 3 zero cols)
P = 128
NBT = B // P  # 4 batch tiles

CHUNKS = [1021, 1021, 1021, 1021]  # scan chunk widths along T (sum = NLOC)
assert sum(CHUNKS) == NLOC
COFF = [sum(CHUNKS[:i]) for i in range(len(CHUNKS))]
NSC = len(CHUNKS)
FSMAX = max(CHUNKS)
WTMAX = FSMAX + WIN + 1  # input cols per scan chunk (incl. leading zero/halo)
LEAD = WIN  # zero-prefix cols: scans self-seed from initial=0 over the prefix

SL = 512  # psum-bank slice for matmuls
PIECE = 511  # store piece width (1 psum bank, 1 relu + 1 DMA per piece)
NSL = (NLOC + SL - 1) // SL  # 8 slices (last = 500)

f32 = mybir.dt.float32
bf16 = mybir.dt.bfloat16
AOT = mybir.ActivationFunctionType
ALU = mybir.AluOpType
AXL = mybir.AxisListType

# engine-split knobs (fraction of columns handled by the DVE engine; the
# rest goes to Pool for SBUF ops / Act for PSUM-reading relu)
U_DVE = 0.0      # u = s1*s2
M0_DVE = 1.0     # m0 = r1*r2
COV_DVE = 0.0    # cov = s12w - u
CORR_DVE = 1.0   # corr = cov*m0
RELU_ACT = 0.70  # relu: Act share (rest DVE tensor_scalar_max; Pool can't read PSUM)
X1S_ACT = 0.5    # x1s = w*x1: Act share (rest Pool tensor_scalar)
PSL = 1021       # slice width for Pool/DVE post ops (latency hiding)


def _slices(total, step):
    return [(i, min(i + step, total)) for i in range(0, total, step)]


def _kernel_body(tc, out, xs, wconst):
    nc = tc.nc
    import contextlib

    ctx = contextlib.ExitStack()
    with ctx:
        const_pool = ctx.enter_context(tc.tile_pool(name="const", bufs=1))
        in_pool = ctx.enter_context(tc.tile_pool(name="scanin", bufs=4))
        s_pool = ctx.enter_context(tc.tile_pool(name="scanout", bufs=2))
        post_pool = ctx.enter_context(tc.tile_pool(name="post", bufs=2))
        corr_pool = ctx.enter_context(tc.tile_pool(name="corrp", bufs=2))
        row_pool = ctx.enter_context(tc.tile_pool(name="rows", bufs=2))
        out_pool = ctx.enter_context(tc.tile_pool(name="outp", bufs=6))
        v_psum = ctx.enter_context(tc.tile_pool(name="vps", bufs=2, space="PSUM"))
        b_psum = ctx.enter_context(tc.tile_pool(name="bps", bufs=1, space="PSUM"))
        f_psum = ctx.enter_context(tc.tile_pool(name="fps", bufs=2, space="PSUM"))

        # constants: I and -I (bf16, exact); DMAs issued after the first
        # input prefetches (see below) so they don't delay the pipeline head
        identP = const_pool.tile([P, P], bf16, tag="identP")
        identN = const_pool.tile([P, P], bf16, tag="identN")
        ident = [identP, identN]
        bcol = const_pool.tile([P, 1], bf16, tag="bcol")
        nc.vector.memset(bcol[:], 1.0 / B)
        negrow = const_pool.tile([1, P], bf16, tag="negrow")
        nc.vector.memset(negrow[:], -1.0)

        SQW = float(np.sqrt(WIN))

        # column-major: each T-chunk is fully independent (scan seeds come
        # from a reduce over the chunk's own halo). The (c, bt) tasks are
        # software-pipelined with a one-stage skew so every engine queue sees
        # task k+1's scan-stage ops before task k's post-stage ops — the
        # in-order queues never stall behind the cross-engine post chain.
        def dma_stage(c, bt):
            fs = CHUNKS[c]
            c0 = COFF[c]
            wt = fs + WIN + 1
            b0 = bt * P
            x12z = in_pool.tile([P, CH, LEAD + WTMAX], f32, tag="x12z")
            if c == 0:
                # col LEAD ("a[-1]") is still zero from the one-time prefix
                # memset: chunk-0 tasks use each buffer's first rotation
                nc.sync.dma_start(
                    out=x12z[:, :, LEAD + 1 : LEAD + wt],
                    in_=xs[b0 : b0 + P, :, 0 : wt - 1],
                )
            else:
                # halo: col LEAD+j holds x[c0-1+j]
                nc.sync.dma_start(
                    out=x12z[:, :, LEAD : LEAD + wt - 1],
                    in_=xs[b0 : b0 + P, :, c0 - 1 : c0 - 1 + wt - 1],
                )
            return x12z

        def scan_stage(c, bt, x12z):
            fs = CHUNKS[c]
            wt = fs + WIN + 1
            e = in_pool.tile([P, CH, LEAD + WTMAX], bf16, tag="e")
            nc.scalar.activation(
                e[:, :, LEAD : LEAD + wt], x12z[:, :, LEAD : LEAD + wt],
                AOT.Square, scale=SQW,
            )
            x1s = in_pool.tile([P, WTMAX], bf16, tag="x1s")
            xcut = int(wt * X1S_ACT)
            if xcut > 0:
                nc.scalar.mul(
                    x1s[:, 0:xcut], x12z[:, 0, LEAD : LEAD + xcut], float(WIN)
                )
            if xcut < wt:
                nc.gpsimd.tensor_scalar_mul(
                    x1s[:, xcut:wt], x12z[:, 0, LEAD + xcut : LEAD + wt], float(WIN)
                )
            e12 = in_pool.tile([P, LEAD + WTMAX], bf16, tag="e12")
            nc.gpsimd.tensor_mul(
                e12[:, LEAD : LEAD + wt], x1s[:, 0:wt], x12z[:, 1, LEAD : LEAD + wt]
            )

            s_pair = s_pool.tile([P, CH, LEAD + FSMAX], bf16, tag="s_pair")
            se_pair = s_pool.tile([P, CH, LEAD + FSMAX], bf16, tag="se_pair")
            s12 = s_pool.tile([P, LEAD + FSMAX], f32, tag="s12")

            def wsum(dst, src, ch):
                # scan over the zero prefix + data: the first LEAD outputs
                # build the opening window from initial=0 (the subtracted
                # stream reads prefix zeros), so no seed reduce is needed.
                # dst[:, LEAD+k] = windowed sum at c0+k.
                n = LEAD + fs
                if ch is not None:
                    d0, d1 = src[:, ch, LEAD : LEAD + n], src[:, ch, 0:n]
                else:
                    d0, d1 = src[:, LEAD : LEAD + n], src[:, 0:n]
                nc.vector.tensor_tensor_scan(
                    out=dst,
                    data0=d0,
                    data1=d1,
                    initial=0.0,
                    op0=ALU.add,
                    op1=ALU.subtract,
                )

            wsum(s_pair[:, 0, 0 : LEAD + fs], x12z, 0)
            wsum(s_pair[:, 1, 0 : LEAD + fs], x12z, 1)
            wsum(se_pair[:, 0, 0 : LEAD + fs], e, 0)
            wsum(se_pair[:, 1, 0 : LEAD + fs], e, 1)
            wsum(s12[:, 0 : LEAD + fs], e12, None)
            return s_pair, se_pair, s12

        def split_op(dve_emit, pool_emit, frac, fs):
            # column-split an elementwise op: [0, cut) on DVE, [cut, fs) Pool
            cut = int(fs * frac) // 2 * 2
            for (l, r) in _slices(fs, PSL):
                dl, dr = min(l, cut), min(r, cut)
                if dl < dr:
                    dve_emit(dl, dr)
                pl, pr = max(l, cut), max(r, cut)
                if pl < pr:
                    pool_emit(pl, pr)

        def post_stage(c, bt, scans):
            fs = CHUNKS[c]
            s_pair_f, se_pair_f, s12_f = scans
            s_pair = s_pair_f[:, :, LEAD:]
            se_pair = se_pair_f[:, :, LEAD:]
            s12 = s12_f[:, LEAD:]
            # t = s^2 (plain; the w-scaling lives in e/e12)
            t_pair = post_pool.tile([P, CH, FSMAX], bf16, tag="t_pair")
            nc.scalar.activation(
                t_pair[:, :, 0:fs], s_pair[:, :, 0:fs], AOT.Square
            )
            u = post_pool.tile([P, FSMAX], f32, tag="u")
            split_op(
                lambda l, r: nc.vector.tensor_mul(
                    u[:, l:r], s_pair[:, 0, l:r], s_pair[:, 1, l:r]
                ),
                lambda l, r: nc.gpsimd.tensor_mul(
                    u[:, l:r], s_pair[:, 0, l:r], s_pair[:, 1, l:r]
                ),
                U_DVE, fs,
            )

            # v = w*s11 - s1^2 (PE identity matmuls into PSUM), rsqrt on Act
            r_pair = post_pool.tile([P, CH, FSMAX], bf16, tag="t_pair", name="r_pair")
            for (l, r) in _slices(fs, SL):
                vps = v_psum.tile([P, CH, SL], f32, tag="vps")
                for ch in range(CH):
                    nc.tensor.matmul(
                        vps[:, ch, 0 : r - l], ident[0][:], se_pair[:, ch, l:r],
                        start=True, stop=False,
                    )
                    nc.tensor.matmul(
                        vps[:, ch, 0 : r - l], ident[1][:], t_pair[:, ch, l:r],
                        start=False, stop=True,
                    )
                nc.scalar.activation(
                    r_pair[:, :, l:r], vps[:, :, 0 : r - l], AOT.Abs_reciprocal_sqrt
                )

            # cov = w*s12 - u ; m0 = r1*r2 ; corr = cov*m0
            cov = post_pool.tile([P, FSMAX], bf16, tag="cov")
            split_op(
                lambda l, r: nc.vector.tensor_sub(cov[:, l:r], s12[:, l:r], u[:, l:r]),
                lambda l, r: nc.gpsimd.tensor_sub(cov[:, l:r], s12[:, l:r], u[:, l:r]),
                COV_DVE, fs,
            )
            # reuses u's slot (u is dead after cov) to stay inside SBUF
            m0 = post_pool.tile([P, FSMAX], bf16, tag="u", name="m0")
            split_op(
                lambda l, r: nc.vector.tensor_mul(
                    m0[:, l:r], r_pair[:, 0, l:r], r_pair[:, 1, l:r]
                ),
                lambda l, r: nc.gpsimd.tensor_mul(
                    m0[:, l:r], r_pair[:, 0, l:r], r_pair[:, 1, l:r]
                ),
                M0_DVE, fs,
            )
            corr = corr_pool.tile([P, FSMAX], bf16, tag=f"corr{bt}")
            split_op(
                lambda l, r: nc.vector.tensor_mul(corr[:, l:r], cov[:, l:r], m0[:, l:r]),
                lambda l, r: nc.gpsimd.tensor_mul(corr[:, l:r], cov[:, l:r], m0[:, l:r]),
                CORR_DVE, fs,
            )
            return corr

        def mean_store_stage(c, corrs):
            fs = CHUNKS[c]
            c0 = COFF[c]
            srow = row_pool.tile([1, FSMAX], bf16, tag="srow")
            for (l, r) in _slices(fs, SL):
                bps = b_psum.tile([1, SL], f32, tag="bps")
                for bt in range(NBT):
                    nc.tensor.matmul(
                        bps[:, 0 : r - l], bcol[:], corrs[bt][:, l:r],
                        start=(bt == 0), stop=(bt == NBT - 1),
                    )
                nc.scalar.activation(srow[:, l:r], bps[:, 0 : r - l], AOT.Copy)

            # fixed piece width: small chunks store one piece per batch tile,
            # halving their per-DMA HWDGE overhead in the pipeline tail
            half = PIECE
            hi = 0
            for bt in range(NBT):
                b0 = bt * P
                for h0 in range(0, fs, half):
                    h1 = min(h0 + half, fs)
                    outt = out_pool.tile([P, PIECE], f32, tag="outt")
                    fps = f_psum.tile([P, PIECE], f32, tag="fps")
                    for (l, r) in _slices(h1 - h0, SL):
                        l, r = l + h0, r + h0
                        nc.tensor.matmul(
                            fps[:, l - h0 : r - h0], ident[0][:], corrs[bt][:, l:r],
                            start=True, stop=False,
                        )
                        nc.tensor.matmul(
                            fps[:, l - h0 : r - h0], negrow[:], srow[:, l:r],
                            start=False, stop=True,
                        )
                    if int((hi + 1) * RELU_ACT) > int(hi * RELU_ACT):
                        nc.scalar.activation(
                            outt[:, 0 : h1 - h0], fps[:, 0 : h1 - h0], AOT.Relu
                        )
                    else:
                        nc.vector.tensor_scalar_max(
                            outt[:, 0 : h1 - h0], fps[:, 0 : h1 - h0], 0.0
                        )
                    hi += 1
                    nc.sync.dma_start(
                        out=out[b0 : b0 + P, c0 + h0 : c0 + h1],
                        in_=outt[:, 0 : h1 - h0],
                    )

        # one-time zero prefixes for all rotating input buffers (the scans
        # read [0:LEAD) of every buffer; nothing in the loop writes there)
        for _ in range(4):
            zx = in_pool.tile([P, CH, LEAD + WTMAX], f32, tag="x12z")
            nc.vector.memset(zx[:, :, 0 : LEAD + 1], 0.0)
            ze = in_pool.tile([P, CH, LEAD + WTMAX], bf16, tag="e")
            nc.vector.memset(ze[:, :, 0:LEAD], 0.0)
            z12 = in_pool.tile([P, LEAD + WTMAX], bf16, tag="e12")
            nc.vector.memset(z12[:, 0:LEAD], 0.0)
            zskip = in_pool.tile([P, WTMAX], bf16, tag="x1s")
            nc.vector.memset(zskip[:, 0:1], 0.0)

        tasks = [(c, bt) for c in range(NSC) for bt in range(NBT)]
        xq = {}
        scans_q = {}
        corrs_q = {}
        pending_store = []
        for j in range(min(2, len(tasks))):
            xq[j] = dma_stage(*tasks[j])
        nc.sync.dma_start(out=identP[:], in_=wconst[0, :, :])
        nc.sync.dma_start(out=identN[:], in_=wconst[1, :, :])
        for i in range(len(tasks) + 1):
            if i + 2 < len(tasks):
                xq[i + 2] = dma_stage(*tasks[i + 2])
            if i < len(tasks):
                scans_q[tasks[i]] = scan_stage(*tasks[i], xq.pop(i))
            # mean_store for a finished chunk is emitted one task late so its
            # PE-gated relu/srow ops sit behind the next task's scan ops in
            # every engine queue (no head-of-line blocking at chunk seams)
            if 0 < i <= len(tasks):
                c, bt = tasks[i - 1]
                corrs_q[(c, bt)] = post_stage(c, bt, scans_q.pop(tasks[i - 1]))
                if bt == NBT - 1:
                    mean_store_stage(c, [corrs_q.pop((c, b)) for b in range(NBT)])


def build_nc():
    from concourse import bacc

    nc = bacc.Bacc("TRN2", target_bir_lowering=False, debug=False, num_devices=NCORES)
    xs = nc.dram_tensor("xs", [B, CH, FIN], f32, kind="ExternalInput").ap()
    wconst = nc.dram_tensor("wconst", [CH, P, P], bf16, kind="ExternalInput").ap()
    out = nc.dram_tensor("out", [B, NLOC], f32, kind="ExternalOutput").ap()
    with tile.TileContext(nc) as tc:
        _kernel_body(tc, out, xs, wconst)
    nc.compile()
    return nc


_NC = None


def _get_nc():
    global _NC
    if _NC is None:
        _NC = build_nc()
    return _NC


def make_in_maps(x):
    import ml_dtypes

    x = np.asarray(x, dtype=np.float32)
    xpad = np.zeros((B, CH, TPAD), dtype=np.float32)
    xpad[:, :, :T] = x
    eye = np.eye(P, dtype=np.float32)
    wconst = np.stack([eye, -eye]).astype(ml_dtypes.bfloat16)
    return [
        {
            "xs": np.ascontiguousarray(xpad[:, :, c * NLOC : c * NLOC + FIN]),
            "wconst": wconst,
        }
        for c in range(NCORES)
    ]


def _run(x, **kwargs):
    nc = _get_nc()
    res = run_bass_kernel_spmd(nc, make_in_maps(x), core_ids=list(range(NCORES)), **kwargs)
    outs = [np.asarray(res.results[c]["out"]) for c in range(NCORES)]
    full = np.concatenate(outs, axis=1)[:, :N].astype(np.float32)
    return full, res


def kernel(x):
    full, _ = _run(x)
    return full


# revision 57
# speedup vs baseline: 1.0452x; 1.0017x over previous
"""Trainium2 Bass kernel for sliding-window Pearson correlation attention.

Input  x: [512, 2, 32768] f32.
Output attentions: [512, 32669] f32 = relu(corr - mean_b(corr)) where corr is
the per-batch sliding-window (w=100) Pearson correlation of the two channels.

Sharding: split the T/output dimension across the 8 cores (4084 output
columns each, + 99-column halo on the input). Every core sees all 512
batches, so the batch-mean is computed locally - no collective needed.

The T range is cut into 4 independent column chunks; each (chunk, batch
tile) task is software-pipelined with a one-stage skew and 2-deep DMA
prefetch so the in-order engine queues never stall behind the cross-engine
post chain. Per task:
  scans   DVE   5 windowed sums via tensor_tensor_scan (f32 in, bf16 out;
                s12/u in f32). Self-seeding: each input tile carries a
                100-col zero prefix (zeroed once per buffer), so the scan
                builds its opening window from initial=0 - no seed reduces
  squares Act   e = w*x^2, t = s^2 (scale folded into the activation)
  x1s     A/P   w*x1 (split Act copy-scale / Pool tensor_scalar)
  e12     Pool  x1s*x2 tensor_tensor
  u       Pool  s1*s2 (f32 out - Pool rates are dtype-blind)
  v1,v2   PE    identity-matmul accumulate: v = I@s11 - I@t into PSUM
                (weights are free, so the subtract rides the matmul)
  r12     Act   Abs_reciprocal_sqrt(v) from PSUM - one act table covers
                Square/Copy/Relu/AbsRsqrt so only one table load total
  cov     Pool  s12 - u ; m0 = r1*r2 and corr = cov*m0 on DVE (bf16 2x)
  mean    PE    (1/B)ones-matmul accumulated over the 4 batch tiles
  fsub    PE    I@corr - broadcast(avg) into PSUM
  relu    A/D   psum -> sbuf f32 (Act Relu / DVE max, interleaved), DMA out
Hardware-ISA notes baked in: Pool/GPSIMD cannot touch PSUM and rejects
scalar_tensor_tensor and tensor_tensor divide/max; there is no TT divide on
any engine (hence rsqrt+multiplies); fp32r matmul needs rounded producers.
"""

import numpy as np

import concourse.bass as bass
import concourse.mybir as mybir
import concourse.tile as tile
from concourse.bass_utils import run_bass_kernel_spmd

WIN = 100
B = 512
CH = 2
T = 32768
N = T - WIN + 1  # 32669
NCORES = 8
NLOC = 4084  # output columns per core (8*4084 = 32672 >= N; tail dropped)
FIN = NLOC + WIN - 1  # 4183 input columns per core
TPAD = (NCORES - 1) * NLOC + FIN  # 32771 (input padded with 3 zero cols)
P = 128
NBT = B // P  # 4 batch tiles

CHUNKS = [1021, 1021, 1021, 1021]  # scan chunk widths along T (sum = NLOC)
assert sum(CHUNKS) == NLOC
COFF = [sum(CHUNKS[:i]) for i in range(len(CHUNKS))]
NSC = len(CHUNKS)
FSMAX = max(CHUNKS)
WTMAX = FSMAX + WIN + 1  # input cols per scan chunk (incl. leading zero/halo)
LEAD = WIN  # zero-prefix cols: scans self-seed from initial=0 over the prefix

SL = 512  # psum-bank slice for matmuls
PIECE = 511  # store piece width (1 psum bank, 1 relu + 1 DMA per piece)
NSL = (NLOC + SL - 1) // SL  # 8 slices (last = 500)

f32 = mybir.dt.float32
bf16 = mybir.dt.bfloat16
AOT = mybir.ActivationFunctionType
ALU = mybir.AluOpType
AXL = mybir.AxisListType

# engine-split knobs (fraction of columns handled by the DVE engine; the
# rest goes to Pool for SBUF ops / Act for PSUM-reading relu)
U_DVE = 0.0      # u = s1*s2
M0_DVE = 1.0     # m0 = r1*r2
COV_DVE = 0.0    # cov = s12w - u
CORR_DVE = 1.0   # corr = cov*m0
RELU_ACT = 0.70  # relu: Act share (rest DVE tensor_scalar_max; Pool can't read PSUM)
X1S_ACT = 0.5    # x1s = w*x1: Act share (rest Pool tensor_scalar)
PSL = 1021       # slice width for Pool/DVE post ops (latency hiding)


def _slices(total, step):
    return [(i, min(i + step, total)) for i in range(0, total, step)]


def _kernel_body(tc, out, xs, wconst):
    nc = tc.nc
    import contextlib

    ctx = contextlib.ExitStack()
    with ctx:
        const_pool = ctx.enter_context(tc.tile_pool(name="const", bufs=1))
        in_pool = ctx.enter_context(tc.tile_pool(name="scanin", bufs=4))
        s_pool = ctx.enter_context(tc.tile_pool(name="scanout", bufs=2))
        post_pool = ctx.enter_context(tc.tile_pool(name="post", bufs=2))
        corr_pool = ctx.enter_context(tc.tile_pool(name="corrp", bufs=2))
        row_pool = ctx.enter_context(tc.tile_pool(name="rows", bufs=2))
        out_pool = ctx.enter_context(tc.tile_pool(name="outp", bufs=6))
        v_psum = ctx.enter_context(tc.tile_pool(name="vps", bufs=2, space="PSUM"))
        b_psum = ctx.enter_context(tc.tile_pool(name="bps", bufs=1, space="PSUM"))
        f_psum = ctx.enter_context(tc.tile_pool(name="fps", bufs=2, space="PSUM"))

        # constants: I and -I (bf16, exact); DMAs issued after the first
        # input prefetches (see below) so they don't delay the pipeline head
        identP = const_pool.tile([P, P], bf16, tag="identP")
        identN = const_pool.tile([P, P], bf16, tag="identN")
        ident = [identP, identN]
        bcol = const_pool.tile([P, 1], bf16, tag="bcol")
        nc.vector.memset(bcol[:], 1.0 / B)
        negrow = const_pool.tile([1, P], bf16, tag="negrow")
        nc.vector.memset(negrow[:], -1.0)

        SQW = float(np.sqrt(WIN))

        # column-major: each T-chunk is fully independent (scan seeds come
        # from a reduce over the chunk's own halo). The (c, bt) tasks are
        # software-pipelined with a one-stage skew so every engine queue sees
        # task k+1's scan-stage ops before task k's post-stage ops — the
        # in-order queues never stall behind the cross-engine post chain.
        def dma_stage(c, bt):
            fs = CHUNKS[c]
            c0 = COFF[c]
            wt = fs + WIN + 1
            b0 = bt * P
            x12z = in_pool.tile([P, CH, LEAD + WTMAX], f32, tag="x12z")
            if c == 0:
                # col LEAD ("a[-1]") is still zero from the one-time prefix
                # memset: chunk-0 tasks use each buffer's first rotation
                nc.sync.dma_start(
                    out=x12z[:, :, LEAD + 1 : LEAD + wt],
                    in_=xs[b0 : b0 + P, :, 0 : wt - 1],
                )
            else:
                # halo: col LEAD+j holds x[c0-1+j]
                nc.sync.dma_start(
                    out=x12z[:, :, LEAD : LEAD + wt - 1],
                    in_=xs[b0 : b0 + P, :, c0 - 1 : c0 - 1 + wt - 1],
                )
            return x12z

        def scan_stage(c, bt, x12z):
            fs = CHUNKS[c]
            wt = fs + WIN + 1
            e = in_pool.tile([P, CH, LEAD + WTMAX], bf16, tag="e")
            nc.scalar.activation(
                e[:, :, LEAD : LEAD + wt], x12z[:, :, LEAD : LEAD + wt],
                AOT.Square, scale=SQW,
            )
            x1s = in_pool.tile([P, WTMAX], bf16, tag="x1s")
            xcut = int(wt * X1S_ACT)
            if xcut > 0:
                nc.scalar.mul(
                    x1s[:, 0:xcut], x12z[:, 0, LEAD : LEAD + xcut], float(WIN)
                )
            if xcut < wt:
                nc.gpsimd.tensor_scalar_mul(
                    x1s[:, xcut:wt], x12z[:, 0, LEAD + xcut : LEAD + wt], float(WIN)
                )
            e12 = in_pool.tile([P, LEAD + WTMAX], bf16, tag="e12")
            nc.gpsimd.tensor_mul(
                e12[:, LEAD : LEAD + wt], x1s[:, 0:wt], x12z[:, 1, LEAD : LEAD + wt]
            )

            s_pair = s_pool.tile([P, CH, LEAD + FSMAX], bf16, tag="s_pair")
            se_pair = s_pool.tile([P, CH, LEAD + FSMAX], bf16, tag="se_pair")
            s12 = s_pool.tile([P, LEAD + FSMAX], f32, tag="s12")

            def wsum(dst, src, ch):
                # scan over the zero prefix + data: the first LEAD outputs
                # build the opening window from initial=0 (the subtracted
                # stream reads prefix zeros), so no seed reduce is needed.
                # dst[:, LEAD+k] = windowed sum at c0+k.
                n = LEAD + fs
                if ch is not None:
                    d0, d1 = src[:, ch, LEAD : LEAD + n], src[:, ch, 0:n]
                else:
                    d0, d1 = src[:, LEAD : LEAD + n], src[:, 0:n]
                nc.vector.tensor_tensor_scan(
                    out=dst,
                    data0=d0,
                    data1=d1,
                    initial=0.0,
                    op0=ALU.add,
                    op1=ALU.subtract,
                )

            wsum(s_pair[:, 0, 0 : LEAD + fs], x12z, 0)
            wsum(s_pair[:, 1, 0 : LEAD + fs], x12z, 1)
            wsum(se_pair[:, 0, 0 : LEAD + fs], e, 0)
            wsum(se_pair[:, 1, 0 : LEAD + fs], e, 1)
            wsum(s12[:, 0 : LEAD + fs], e12, None)
            return s_pair, se_pair, s12

        def split_op(dve_emit, pool_emit, frac, fs):
            # column-split an elementwise op: [0, cut) on DVE, [cut, fs) Pool
            # (frac=1 must mean exactly no Pool part: the rounded-down cut
            # would otherwise leave a 1-col Pool sliver whose dependency edge
            # gates the downstream mean matmuls)
            cut = fs if frac >= 1.0 else int(fs * frac) // 2 * 2
            for (l, r) in _slices(fs, PSL):
                dl, dr = min(l, cut), min(r, cut)
                if dl < dr:
                    dve_emit(dl, dr)
                pl, pr = max(l, cut), max(r, cut)
                if pl < pr:
                    pool_emit(pl, pr)

        def post_stage(c, bt, scans):
            fs = CHUNKS[c]
            s_pair_f, se_pair_f, s12_f = scans
            s_pair = s_pair_f[:, :, LEAD:]
            se_pair = se_pair_f[:, :, LEAD:]
            s12 = s12_f[:, LEAD:]
            # t = s^2 (plain; the w-scaling lives in e/e12)
            t_pair = post_pool.tile([P, CH, FSMAX], bf16, tag="t_pair")
            nc.scalar.activation(
                t_pair[:, :, 0:fs], s_pair[:, :, 0:fs], AOT.Square
            )
            u = post_pool.tile([P, FSMAX], f32, tag="u")
            split_op(
                lambda l, r: nc.vector.tensor_mul(
                    u[:, l:r], s_pair[:, 0, l:r], s_pair[:, 1, l:r]
                ),
                lambda l, r: nc.gpsimd.tensor_mul(
                    u[:, l:r], s_pair[:, 0, l:r], s_pair[:, 1, l:r]
                ),
                U_DVE, fs,
            )

            # v = w*s11 - s1^2 (PE identity matmuls into PSUM), rsqrt on Act
            r_pair = post_pool.tile([P, CH, FSMAX], bf16, tag="t_pair", name="r_pair")
            for (l, r) in _slices(fs, SL):
                vps = v_psum.tile([P, CH, SL], f32, tag="vps")
                for ch in range(CH):
                    nc.tensor.matmul(
                        vps[:, ch, 0 : r - l], ident[0][:], se_pair[:, ch, l:r],
                        start=True, stop=False,
                    )
                    nc.tensor.matmul(
                        vps[:, ch, 0 : r - l], ident[1][:], t_pair[:, ch, l:r],
                        start=False, stop=True,
                    )
                nc.scalar.activation(
                    r_pair[:, :, l:r], vps[:, :, 0 : r - l], AOT.Abs_reciprocal_sqrt
                )

            # cov = w*s12 - u ; m0 = r1*r2 ; corr = cov*m0
            cov = post_pool.tile([P, FSMAX], bf16, tag="cov")
            split_op(
                lambda l, r: nc.vector.tensor_sub(cov[:, l:r], s12[:, l:r], u[:, l:r]),
                lambda l, r: nc.gpsimd.tensor_sub(cov[:, l:r], s12[:, l:r], u[:, l:r]),
                COV_DVE, fs,
            )
            # reuses u's slot (u is dead after cov) to stay inside SBUF
            m0 = post_pool.tile([P, FSMAX], bf16, tag="u", name="m0")
            split_op(
                lambda l, r: nc.vector.tensor_mul(
                    m0[:, l:r], r_pair[:, 0, l:r], r_pair[:, 1, l:r]
                ),
                lambda l, r: nc.gpsimd.tensor_mul(
                    m0[:, l:r], r_pair[:, 0, l:r], r_pair[:, 1, l:r]
                ),
                M0_DVE, fs,
            )
            corr = corr_pool.tile([P, FSMAX], bf16, tag=f"corr{bt}")
            split_op(
                lambda l, r: nc.vector.tensor_mul(corr[:, l:r], cov[:, l:r], m0[:, l:r]),
                lambda l, r: nc.gpsimd.tensor_mul(corr[:, l:r], cov[:, l:r], m0[:, l:r]),
                CORR_DVE, fs,
            )
            return corr

        def mean_store_stage(c, corrs):
            fs = CHUNKS[c]
            c0 = COFF[c]
            srow = row_pool.tile([1, FSMAX], bf16, tag="srow")
            for (l, r) in _slices(fs, SL):
                bps = b_psum.tile([1, SL], f32, tag="bps")
                for bt in range(NBT):
                    nc.tensor.matmul(
                        bps[:, 0 : r - l], bcol[:], corrs[bt][:, l:r],
                        start=(bt == 0), stop=(bt == NBT - 1),
                    )
                nc.scalar.activation(srow[:, l:r], bps[:, 0 : r - l], AOT.Copy)

            # fixed piece width: small chunks store one piece per batch tile,
            # halving their per-DMA HWDGE overhead in the pipeline tail
            half = PIECE
            hi = 0
            for bt in range(NBT):
                b0 = bt * P
                for h0 in range(0, fs, half):
                    h1 = min(h0 + half, fs)
                    outt = out_pool.tile([P, PIECE], f32, tag="outt")
                    fps = f_psum.tile([P, PIECE], f32, tag="fps")
                    for (l, r) in _slices(h1 - h0, SL):
                        l, r = l + h0, r + h0
                        nc.tensor.matmul(
                            fps[:, l - h0 : r - h0], ident[0][:], corrs[bt][:, l:r],
                            start=True, stop=False,
                        )
                        nc.tensor.matmul(
                            fps[:, l - h0 : r - h0], negrow[:], srow[:, l:r],
                            start=False, stop=True,
                        )
                    if int((hi + 1) * RELU_ACT) > int(hi * RELU_ACT):
                        nc.scalar.activation(
                            outt[:, 0 : h1 - h0], fps[:, 0 : h1 - h0], AOT.Relu
                        )
                    else:
                        nc.vector.tensor_scalar_max(
                            outt[:, 0 : h1 - h0], fps[:, 0 : h1 - h0], 0.0
                        )
                    hi += 1
                    nc.sync.dma_start(
                        out=out[b0 : b0 + P, c0 + h0 : c0 + h1],
                        in_=outt[:, 0 : h1 - h0],
                    )

        # one-time zero prefixes for all rotating input buffers (the scans
        # read [0:LEAD) of every buffer; nothing in the loop writes there)
        for _ in range(4):
            zx = in_pool.tile([P, CH, LEAD + WTMAX], f32, tag="x12z")
            nc.vector.memset(zx[:, :, 0 : LEAD + 1], 0.0)
            ze = in_pool.tile([P, CH, LEAD + WTMAX], bf16, tag="e")
            nc.vector.memset(ze[:, :, 0:LEAD], 0.0)
            z12 = in_pool.tile([P, LEAD + WTMAX], bf16, tag="e12")
            nc.vector.memset(z12[:, 0:LEAD], 0.0)
            zskip = in_pool.tile([P, WTMAX], bf16, tag="x1s")
            nc.vector.memset(zskip[:, 0:1], 0.0)

        tasks = [(c, bt) for c in range(NSC) for bt in range(NBT)]
        xq = {}
        scans_q = {}
        corrs_q = {}
        pending_store = []
        for j in range(min(2, len(tasks))):
            xq[j] = dma_stage(*tasks[j])
        nc.sync.dma_start(out=identP[:], in_=wconst[0, :, :])
        nc.sync.dma_start(out=identN[:], in_=wconst[1, :, :])
        for i in range(len(tasks) + 1):
            if i + 2 < len(tasks):
                xq[i + 2] = dma_stage(*tasks[i + 2])
            if i < len(tasks):
                scans_q[tasks[i]] = scan_stage(*tasks[i], xq.pop(i))
            # mean_store for a finished chunk is emitted one task late so its
            # PE-gated relu/srow ops sit behind the next task's scan ops in
            # every engine queue (no head-of-line blocking at chunk seams)
            if 0 < i <= len(tasks):
                c, bt = tasks[i - 1]
                corrs_q[(c, bt)] = post_stage(c, bt, scans_q.pop(tasks[i - 1]))
                if bt == NBT - 1:
                    mean_store_stage(c, [corrs_q.pop((c, b)) for b in range(NBT)])


def build_nc():
    from concourse import bacc

    nc = bacc.Bacc("TRN2", target_bir_lowering=False, debug=False, num_devices=NCORES)
    xs = nc.dram_tensor("xs", [B, CH, FIN], f32, kind="ExternalInput").ap()
    wconst = nc.dram_tensor("wconst", [CH, P, P], bf16, kind="ExternalInput").ap()
    out = nc.dram_tensor("out", [B, NLOC], f32, kind="ExternalOutput").ap()
    with tile.TileContext(nc) as tc:
        _kernel_body(tc, out, xs, wconst)
    nc.compile()
    return nc


_NC = None


def _get_nc():
    global _NC
    if _NC is None:
        _NC = build_nc()
    return _NC


def make_in_maps(x):
    import ml_dtypes

    x = np.asarray(x, dtype=np.float32)
    xpad = np.zeros((B, CH, TPAD), dtype=np.float32)
    xpad[:, :, :T] = x
    eye = np.eye(P, dtype=np.float32)
    wconst = np.stack([eye, -eye]).astype(ml_dtypes.bfloat16)
    return [
        {
            "xs": np.ascontiguousarray(xpad[:, :, c * NLOC : c * NLOC + FIN]),
            "wconst": wconst,
        }
        for c in range(NCORES)
    ]


def _run(x, **kwargs):
    nc = _get_nc()
    res = run_bass_kernel_spmd(nc, make_in_maps(x), core_ids=list(range(NCORES)), **kwargs)
    outs = [np.asarray(res.results[c]["out"]) for c in range(NCORES)]
    full = np.concatenate(outs, axis=1)[:, :N].astype(np.float32)
    return full, res


def kernel(x):
    full, _ = _run(x)
    return full
